# revision 18
# baseline (speedup 1.0000x reference)
"""AdaptiveCLPL loss on 8 TRN2 NeuronCores (Bass/Tile), v4.

loss = mean_b [ psi(avg_cand_b) + sum_head psi(-l)(1-mask) + ts*sum_samp psi(-l)(1-iscand) ]
psi(u) = softplus(-u); psi(-l) = softplus(l) = Ln(Exp(l)+1) (composite; both
funcs live in the single natural_log_exp_and_others act table -> one load).

Decomposition (host does index-driven data selection/layout only; every
logit VALUE is read, transformed and reduced on device):
  total = sum_b softplus(-avg_b)                       [term1]
        + sum_{head block} softplus(l)                 [bulk DMA + ACT/DVE]
        + ts * sum_{sampled cols, all rows} softplus(l)
        + sum_cand wcorr * softplus(l_cand),  wcorr = -uniq*(inhead + ts*smult)

Per-core layout (rows = its 256-row batch shard; batch row b lives at
partition p=b%128, group g=b//128):
- "pref" [128, 2*GMX + 2*NCP + 2*S] bf16:
    [cand g0 | cand g1]: row (p,g)'s unique non-correction candidate logits,
      padded with 0.0 (adds zero to the row-sum) -> csum = plain row reduce.
    [corr g0 | corr g1]: correction-candidate logits at their row's slot;
      also reduced into csum; softplus(pad=0)=ln2 is cancelled by wcpm=0.
    [samp h0 | samp h1]: the 100 sampled columns for all 256 rows
      (every element needed) -> softplus + accum.
- "lTh" [2000, 256] bf16 head block, viewed [125, 4096]: chunked ACT
  Exp then Ln(+1) with per-chunk row-sum accumulation (overlaps the DMA).
Per-core [128,1] partials are summed on host (no collectives, no gpsimd,
no SWDGE gathers -- see kernel_gather.py for why gathers lose: ~8.4ns/idx
serial descriptor emission + ~9us IRAM library load + first-run races).
"""

import numpy as np
import ml_dtypes

B, C, K = 2048, 50000, 10
HEAD, S = 2000, 100
TSCALE = float(C - HEAD) / float(S)  # 480.0
NCORES = 8
RB = B // NCORES   # 256
P = 128
HP = 125           # head tile partitions; 2000 = 125*16
HB = HEAD // HP    # 16
HW4 = HB * RB // 4  # head tile quarter width (1024)
HEADP = 2048       # head cols padded to 128*16 (pad value -20)
HBP = HEADP // P   # 16
BF16 = ml_dtypes.bfloat16

_CACHE = {}


def prep_inputs(logits, candidates, sampled_indices):
    logits = np.asarray(logits)
    candidates = np.asarray(candidates)
    sampled_indices = np.asarray(sampled_indices)
    assert logits.shape == (B, C) and candidates.shape == (B, K)
    srow = (HEAD + sampled_indices.astype(np.int64))      # [S] column ids
    svals, scounts = np.unique(srow, return_counts=True)
    smult_map = dict(zip(svals.tolist(), scounts.tolist()))

    cores = []
    for i in range(NCORES):
        rows = slice(i * RB, (i + 1) * RB)
        cand = candidates[rows].astype(np.int64)
        valid = cand >= 0
        uniq = valid.copy()
        for k in range(1, K):
            dup = (cand[:, :k] == cand[:, k:k + 1]).any(axis=1)
            uniq[:, k] &= ~dup
        cnt = np.maximum(uniq.sum(axis=1), 1).astype(np.float32)
        inhead = cand < HEAD
        mult = np.vectorize(lambda c: smult_map.get(int(c), 0))(cand)
        iscorr = uniq & (inhead | (mult > 0))
        plain = [[] for _ in range(RB)]   # candidate col ids per row
        corr = [[] for _ in range(RB)]    # (col, wcorr) per row
        for b in range(RB):
            for k in range(K):
                if not uniq[b, k]:
                    continue
                if iscorr[b, k]:
                    corr[b].append((int(cand[b, k]),
                                    -(float(inhead[b, k])
                                      + TSCALE * float(mult[b, k]))))
                else:
                    plain[b].append(int(cand[b, k]))
        cores.append((plain, corr, cnt))

    gmx = max(max(len(pl) for pl in plain_) or 1
              for plain_, _, _ in cores)
    ncp = max(max(len(co) for co in corr_) or 1
              for _, corr_, _ in cores)
    plan = (gmx, ncp)

    in_maps = []
    for i in range(NCORES):
        plain, corr, cnt = cores[i]
        rows = slice(i * RB, (i + 1) * RB)
        lrows = logits[rows]                              # [256, C] f32

        cv = np.zeros((RB, gmx), np.float32)
        xv = np.zeros((RB, ncp), np.float32)
        wc = np.zeros((RB, ncp), np.float32)
        for b in range(RB):
            for j, col in enumerate(plain[b]):
                cv[b, j] = lrows[b, col]
            for j, (col, w) in enumerate(corr[b]):
                xv[b, j] = lrows[b, col]
                wc[b, j] = w
        sampv = lrows[:, srow]                            # [256, S]

        def fold(a):
            """[256, W] -> [128, 2*W] with (p, g*W + j) = a[g*128+p, j]."""
            return np.concatenate([a[:128], a[128:]], axis=1)

        rcnt = np.zeros((P, 2), np.float32)
        for b in range(RB):
            rcnt[b % 128, b // 128] = 1.0 / cnt[b]
        # head padded to 2048 columns with -20 (softplus ~ 2e-9): partition
        # p holds cols [16p, 16p+16) contiguously -> [128, 4096]
        hpad = np.full((HEADP, RB), -20.0, np.float32)
        hpad[:HEAD] = lrows[:, :HEAD].T
        lTh = hpad.astype(BF16).reshape(P, HBP * RB)
        smalls = np.concatenate(
            [fold(cv), fold(xv), fold(sampv), fold(wc), rcnt], axis=1)
        spad = np.zeros((P, (-smalls.shape[1]) % 256), np.float32)
        pref = np.concatenate([smalls, spad, lTh], axis=1).astype(BF16)

        in_maps.append({
            "pref": np.ascontiguousarray(pref),
        })
    return in_maps, plan


def _build(plan, enable_asserts=False):
    import concourse.tile as tile
    from concourse import bacc, mybir

    gmx, ncp = plan
    SW0 = 2 * gmx + 2 * ncp + 2 * S + 2 * ncp + 2
    SW = SW0 + ((-SW0) % 256)
    PW = SW + HBP * RB

    f32 = mybir.dt.float32
    bf16 = mybir.dt.bfloat16
    AF = mybir.ActivationFunctionType
    OP = mybir.AluOpType
    AX = mybir.AxisListType

    nc = bacc.Bacc("TRN2", target_bir_lowering=False, debug=False,
                   enable_asserts=enable_asserts, num_devices=NCORES)

    from concourse.hw_specs import get_activation_tables
    tabs = get_activation_tables(nc.m.arch)
    if "natural_log_exp_and_others" in tabs:
        for nm, funcs in tabs.items():
            if nm != "natural_log_exp_and_others":
                funcs.discard(AF.Exp)
                funcs.discard(AF.Ln)

    pref = nc.dram_tensor("pref", [P, PW], bf16, kind="ExternalInput").ap()
    out = nc.dram_tensor("out", [P, 6], f32, kind="ExternalOutput").ap()


    with tile.TileContext(nc) as tc:
        with tc.tile_pool(name="sb", bufs=1) as sb:
            # --- ONE packed input, two half-transfers (fixed DMA cost
            # ~2.5us dominates small transfers; payload ~285GB/s) ---
            pf = sb.tile([P, PW], bf16)
            halfc = PW // 2
            nc.sync.dma_start(out=pf[:, :halfc], in_=pref[:, :halfc])
            nc.scalar.dma_start(out=pf[:, halfc:], in_=pref[:, halfc:])
            ht = pf[:, SW:PW]

            o = 0
            cand_t = pf[:, o:o + 2 * gmx]; o += 2 * gmx
            corr_t = pf[:, o:o + 2 * ncp]; o += 2 * ncp
            samp_t = pf[:, o:o + 2 * S]; o += 2 * S
            wcpm_b = pf[:, o:o + 2 * ncp]; o += 2 * ncp
            rcnt_b = pf[:, o:o + 2]; o += 2
            # f32 working copies of the bf16 weights
            wr = sb.tile([P, 2 * ncp + 2], f32)
            nc.vector.tensor_scalar_mul(wr[:, :], pf[:, o - 2 * ncp - 2:o],
                                        1.0)
            wcpm_t = wr[:, 0:2 * ncp]
            rcnt_t = wr[:, 2 * ncp:2 * ncp + 2]

            # --- sampled: softplus + accum (bf16) ---
            sp = sb.tile([P, 2 * S], bf16)
            nc.scalar.activation(sp[:, :], samp_t, AF.Exp)
            sacc = sb.tile([P, 1], f32)
            nc.scalar.activation(sp[:, :], sp[:, :], AF.Ln, bias=1.0,
                                 accum_out=sacc[:, :])

            # --- corrections: softplus(corr values) dot wcpm ---
            ce = sb.tile([P, 2 * ncp], f32)
            nc.scalar.activation(ce[:, :], corr_t, AF.Exp)
            spl = sb.tile([P, 2 * ncp], f32)
            nc.scalar.activation(spl[:, :], ce[:, :], AF.Ln, bias=1.0)
            nc.vector.tensor_tensor(spl[:, :], spl[:, :], wcpm_t,
                                    op=OP.mult)
            corr1 = sb.tile([P, 1], f32)
            nc.vector.tensor_reduce(corr1[:, :], spl[:, :], AX.X, OP.add)

            # --- candidate row sums -> avg -> term1 ---
            csum = sb.tile([P, 2], f32)
            nc.vector.tensor_reduce(
                csum[:, :], cand_t.rearrange("p (g j) -> p g j", g=2),
                AX.X, OP.add)
            csc = sb.tile([P, 2], f32)
            nc.vector.tensor_reduce(
                csc[:, :], corr_t.rearrange("p (g j) -> p g j", g=2),
                AX.X, OP.add)
            nc.vector.tensor_tensor(csum[:, :], csum[:, :], csc[:, :],
                                    op=OP.add)
            avg = sb.tile([P, 2], f32)
            nc.vector.tensor_tensor(avg[:, :], csum[:, :], rcnt_t,
                                    op=OP.mult)
            ae = sb.tile([P, 2], f32)
            nc.scalar.activation(ae[:, :], avg[:, :], AF.Exp, scale=-1.0)
            t1 = sb.tile([P, 2], f32)
            t1c = sb.tile([P, 1], f32)
            nc.scalar.activation(t1[:, :], ae[:, :], AF.Ln, bias=1.0,
                                 accum_out=t1c[:, :])

            # --- head softplus, 2 chunks (in-place on the packed tile) ---
            hacc4 = sb.tile([P, 2], f32)
            hw2 = HBP * RB // 2
            for qi in range(2):
                sl = slice(SW + qi * hw2, SW + (qi + 1) * hw2)
                nc.scalar.activation(pf[:, sl], pf[:, sl], AF.Exp)
                nc.scalar.activation(pf[:, sl], pf[:, sl], AF.Ln, bias=1.0,
                                     accum_out=hacc4[:, qi:qi + 1])

            # --- outputs: small terms early, head accums when done;
            # the host sums the 6 columns ---
            total = sb.tile([P, 6], f32)
            nc.vector.memset(total[:, :], 0.0)
            nc.vector.tensor_scalar_mul(total[:, 0:1], sacc[:, :], TSCALE)
            nc.vector.tensor_tensor(total[:, 0:1], total[:, 0:1],
                                    t1c[:, :], op=OP.add)
            nc.vector.tensor_tensor(total[:, 0:1], total[:, 0:1],
                                    corr1[:, :], op=OP.add)
            nc.sync.dma_start(out=out[:, 0:2], in_=total[:, 0:2])
            nc.vector.tensor_tensor(total[:, 2:4], total[:, 2:4],
                                    hacc4[:, :], op=OP.add)
            nc.scalar.dma_start(out=out[:, 2:4], in_=total[:, 2:4])

    nc.compile()
    return nc


def get_graph(plan, enable_asserts=False):
    key = (plan, enable_asserts)
    if key not in _CACHE:
        _CACHE[key] = _build(plan, enable_asserts=enable_asserts)
    return _CACHE[key]


def run(logits, candidates, sampled_indices, trace=False, **kw):
    from concourse.bass_utils import run_bass_kernel_spmd

    in_maps, plan = prep_inputs(logits, candidates, sampled_indices)
    nc = get_graph(plan)
    res = run_bass_kernel_spmd(nc, in_maps, core_ids=list(range(NCORES)),
                               trace=trace, **kw)
    parts = [r["out"].astype(np.float64).sum() for r in res.results]
    loss = np.float32(sum(parts) / B)
    return loss, res


def kernel(logits, candidates, sampled_indices):
    loss, _ = run(logits, candidates, sampled_indices, trace=False)
    return loss


# revision 19
# speedup vs baseline: 1.0317x; 1.0317x over previous
"""AdaptiveCLPL loss on 8 TRN2 NeuronCores (Bass/Tile), v4.

loss = mean_b [ psi(avg_cand_b) + sum_head psi(-l)(1-mask) + ts*sum_samp psi(-l)(1-iscand) ]
psi(u) = softplus(-u); psi(-l) = softplus(l) = Ln(Exp(l)+1) (composite; both
funcs live in the single natural_log_exp_and_others act table -> one load).

Decomposition (host does index-driven data selection/layout only; every
logit VALUE is read, transformed and reduced on device):
  total = sum_b softplus(-avg_b)                       [term1]
        + sum_{head block} softplus(l)                 [bulk DMA + ACT/DVE]
        + ts * sum_{sampled cols, all rows} softplus(l)
        + sum_cand wcorr * softplus(l_cand),  wcorr = -uniq*(inhead + ts*smult)

Per-core layout (rows = its 256-row batch shard; batch row b lives at
partition p=b%128, group g=b//128):
- "pref" [128, 2*GMX + 2*NCP + 2*S] bf16:
    [cand g0 | cand g1]: row (p,g)'s unique non-correction candidate logits,
      padded with 0.0 (adds zero to the row-sum) -> csum = plain row reduce.
    [corr g0 | corr g1]: correction-candidate logits at their row's slot;
      also reduced into csum; softplus(pad=0)=ln2 is cancelled by wcpm=0.
    [samp h0 | samp h1]: the 100 sampled columns for all 256 rows
      (every element needed) -> softplus + accum.
- "lTh" [2000, 256] bf16 head block, viewed [125, 4096]: chunked ACT
  Exp then Ln(+1) with per-chunk row-sum accumulation (overlaps the DMA).
Per-core [128,1] partials are summed on host (no collectives, no gpsimd,
no SWDGE gathers -- see kernel_gather.py for why gathers lose: ~8.4ns/idx
serial descriptor emission + ~9us IRAM library load + first-run races).
"""

import numpy as np
import ml_dtypes

B, C, K = 2048, 50000, 10
HEAD, S = 2000, 100
TSCALE = float(C - HEAD) / float(S)  # 480.0
NCORES = 8
RB = B // NCORES   # 256
P = 128
HP = 125           # head tile partitions; 2000 = 125*16
HB = HEAD // HP    # 16
HW4 = HB * RB // 4  # head tile quarter width (1024)
HEADP = 2048       # head cols padded to 128*16 (pad value -20)
HBP = HEADP // P   # 16
BF16 = ml_dtypes.bfloat16

_CACHE = {}


def prep_inputs(logits, candidates, sampled_indices):
    logits = np.asarray(logits)
    candidates = np.asarray(candidates)
    sampled_indices = np.asarray(sampled_indices)
    assert logits.shape == (B, C) and candidates.shape == (B, K)
    srow = (HEAD + sampled_indices.astype(np.int64))      # [S] column ids
    svals, scounts = np.unique(srow, return_counts=True)
    smult_map = dict(zip(svals.tolist(), scounts.tolist()))

    cores = []
    for i in range(NCORES):
        rows = slice(i * RB, (i + 1) * RB)
        cand = candidates[rows].astype(np.int64)
        valid = cand >= 0
        uniq = valid.copy()
        for k in range(1, K):
            dup = (cand[:, :k] == cand[:, k:k + 1]).any(axis=1)
            uniq[:, k] &= ~dup
        cnt = np.maximum(uniq.sum(axis=1), 1).astype(np.float32)
        inhead = cand < HEAD
        mult = np.vectorize(lambda c: smult_map.get(int(c), 0))(cand)
        iscorr = uniq & (inhead | (mult > 0))
        plain = [[] for _ in range(RB)]   # candidate col ids per row
        corr = [[] for _ in range(RB)]    # (col, wcorr) per row
        for b in range(RB):
            for k in range(K):
                if not uniq[b, k]:
                    continue
                if iscorr[b, k]:
                    corr[b].append((int(cand[b, k]),
                                    -(float(inhead[b, k])
                                      + TSCALE * float(mult[b, k]))))
                else:
                    plain[b].append(int(cand[b, k]))
        cores.append((plain, corr, cnt))

    gmx = max(max(len(pl) for pl in plain_) or 1
              for plain_, _, _ in cores)
    ncp = max(max(len(co) for co in corr_) or 1
              for _, corr_, _ in cores)
    plan = (gmx, ncp)

    in_maps = []
    for i in range(NCORES):
        plain, corr, cnt = cores[i]
        rows = slice(i * RB, (i + 1) * RB)
        lrows = logits[rows]                              # [256, C] f32

        cv = np.zeros((RB, gmx), np.float32)
        xv = np.zeros((RB, ncp), np.float32)
        wc = np.zeros((RB, ncp), np.float32)
        for b in range(RB):
            for j, col in enumerate(plain[b]):
                cv[b, j] = lrows[b, col]
            for j, (col, w) in enumerate(corr[b]):
                xv[b, j] = lrows[b, col]
                wc[b, j] = w
        sampv = lrows[:, srow]                            # [256, S]

        def fold(a):
            """[256, W] -> [128, 2*W] with (p, g*W + j) = a[g*128+p, j]."""
            return np.concatenate([a[:128], a[128:]], axis=1)

        rcnt = np.zeros((P, 2), np.float32)
        for b in range(RB):
            rcnt[b % 128, b // 128] = 1.0 / cnt[b]
        # head padded to 2048 columns with -20 (softplus ~ 2e-9): partition
        # p holds cols [16p, 16p+16) contiguously -> [128, 4096]
        hpad = np.full((HEADP, RB), -20.0, np.float32)
        hpad[:HEAD] = lrows[:, :HEAD].T
        lTh = hpad.astype(BF16).reshape(P, HBP * RB)
        smalls = np.concatenate(
            [fold(cv), fold(xv), fold(sampv), fold(wc), rcnt], axis=1)
        spad = np.zeros((P, (-smalls.shape[1]) % 256), np.float32)
        pref = np.concatenate([smalls, spad, lTh], axis=1).astype(BF16)

        in_maps.append({
            "pref": np.ascontiguousarray(pref),
        })
    return in_maps, plan


def _build(plan, enable_asserts=False):
    import concourse.tile as tile
    from concourse import bacc, mybir

    gmx, ncp = plan
    SW0 = 2 * gmx + 2 * ncp + 2 * S + 2 * ncp + 2
    SW = SW0 + ((-SW0) % 256)
    PW = SW + HBP * RB

    f32 = mybir.dt.float32
    bf16 = mybir.dt.bfloat16
    AF = mybir.ActivationFunctionType
    OP = mybir.AluOpType
    AX = mybir.AxisListType

    nc = bacc.Bacc("TRN2", target_bir_lowering=False, debug=False,
                   enable_asserts=enable_asserts, num_devices=NCORES)

    from concourse.hw_specs import get_activation_tables
    tabs = get_activation_tables(nc.m.arch)
    if "natural_log_exp_and_others" in tabs:
        for nm, funcs in tabs.items():
            if nm != "natural_log_exp_and_others":
                funcs.discard(AF.Exp)
                funcs.discard(AF.Ln)

    pref = nc.dram_tensor("pref", [P, PW], bf16, kind="ExternalInput").ap()
    out = nc.dram_tensor("out", [P, 6], f32, kind="ExternalOutput").ap()


    with tile.TileContext(nc) as tc:
        with tc.tile_pool(name="sb", bufs=1) as sb:
            # --- ONE packed input, two half-transfers (fixed DMA cost
            # ~2.5us dominates small transfers; payload ~285GB/s) ---
            pf = sb.tile([P, PW], bf16)
            halfc = PW // 2
            nc.sync.dma_start(out=pf[:, :halfc], in_=pref[:, :halfc])
            nc.scalar.dma_start(out=pf[:, halfc:], in_=pref[:, halfc:])
            ht = pf[:, SW:PW]

            o = 0
            cand_t = pf[:, o:o + 2 * gmx]; o += 2 * gmx
            corr_t = pf[:, o:o + 2 * ncp]; o += 2 * ncp
            samp_t = pf[:, o:o + 2 * S]; o += 2 * S
            wcpm_b = pf[:, o:o + 2 * ncp]; o += 2 * ncp
            rcnt_b = pf[:, o:o + 2]; o += 2
            # f32 working copies of the bf16 weights
            wr = sb.tile([P, 2 * ncp + 2], f32)
            nc.vector.tensor_scalar_mul(wr[:, :], pf[:, o - 2 * ncp - 2:o],
                                        1.0)
            wcpm_t = wr[:, 0:2 * ncp]
            rcnt_t = wr[:, 2 * ncp:2 * ncp + 2]

            # --- sampled: softplus + accum (bf16) ---
            sp = sb.tile([P, 2 * S], bf16)
            nc.scalar.activation(sp[:, :], samp_t, AF.Exp)
            sacc = sb.tile([P, 1], f32)
            nc.scalar.activation(sp[:, :], sp[:, :], AF.Ln, bias=1.0,
                                 accum_out=sacc[:, :])

            # --- corrections: softplus(corr values) dot wcpm ---
            ce = sb.tile([P, 2 * ncp], f32)
            nc.scalar.activation(ce[:, :], corr_t, AF.Exp)
            spl = sb.tile([P, 2 * ncp], f32)
            nc.scalar.activation(spl[:, :], ce[:, :], AF.Ln, bias=1.0)
            nc.vector.tensor_tensor(spl[:, :], spl[:, :], wcpm_t,
                                    op=OP.mult)
            corr1 = sb.tile([P, 1], f32)
            nc.vector.tensor_reduce(corr1[:, :], spl[:, :], AX.X, OP.add)

            # --- candidate row sums -> avg -> term1 ---
            csum = sb.tile([P, 2], f32)
            nc.vector.tensor_reduce(
                csum[:, :], cand_t.rearrange("p (g j) -> p g j", g=2),
                AX.X, OP.add)
            csc = sb.tile([P, 2], f32)
            nc.vector.tensor_reduce(
                csc[:, :], corr_t.rearrange("p (g j) -> p g j", g=2),
                AX.X, OP.add)
            nc.vector.tensor_tensor(csum[:, :], csum[:, :], csc[:, :],
                                    op=OP.add)
            avg = sb.tile([P, 2], f32)
            nc.vector.tensor_tensor(avg[:, :], csum[:, :], rcnt_t,
                                    op=OP.mult)
            ae = sb.tile([P, 2], f32)
            nc.scalar.activation(ae[:, :], avg[:, :], AF.Exp, scale=-1.0)
            t1 = sb.tile([P, 2], f32)
            t1c = sb.tile([P, 1], f32)
            nc.scalar.activation(t1[:, :], ae[:, :], AF.Ln, bias=1.0,
                                 accum_out=t1c[:, :])

            # --- head softplus, 4 chunks (in-place on the packed tile) ---
            hacc4 = sb.tile([P, 4], f32)
            hw2 = HBP * RB // 4
            for qi in range(4):
                sl = slice(SW + qi * hw2, SW + (qi + 1) * hw2)
                nc.scalar.activation(pf[:, sl], pf[:, sl], AF.Exp)
                nc.scalar.activation(pf[:, sl], pf[:, sl], AF.Ln, bias=1.0,
                                     accum_out=hacc4[:, qi:qi + 1])

            # --- outputs: small terms early, head accums when done;
            # the host sums the 6 columns ---
            total = sb.tile([P, 6], f32)
            nc.vector.memset(total[:, :], 0.0)
            nc.vector.tensor_scalar_mul(total[:, 0:1], sacc[:, :], TSCALE)
            nc.vector.tensor_tensor(total[:, 0:1], total[:, 0:1],
                                    t1c[:, :], op=OP.add)
            nc.vector.tensor_tensor(total[:, 0:1], total[:, 0:1],
                                    corr1[:, :], op=OP.add)
            nc.sync.dma_start(out=out[:, 0:2], in_=total[:, 0:2])
            nc.vector.tensor_tensor(total[:, 2:6], total[:, 2:6],
                                    hacc4[:, :], op=OP.add)
            nc.scalar.dma_start(out=out[:, 2:6], in_=total[:, 2:6])

    nc.compile()
    return nc


def get_graph(plan, enable_asserts=False):
    key = (plan, enable_asserts)
    if key not in _CACHE:
        _CACHE[key] = _build(plan, enable_asserts=enable_asserts)
    return _CACHE[key]


def run(logits, candidates, sampled_indices, trace=False, **kw):
    from concourse.bass_utils import run_bass_kernel_spmd

    in_maps, plan = prep_inputs(logits, candidates, sampled_indices)
    nc = get_graph(plan)
    res = run_bass_kernel_spmd(nc, in_maps, core_ids=list(range(NCORES)),
                               trace=trace, **kw)
    parts = [r["out"].astype(np.float64).sum() for r in res.results]
    loss = np.float32(sum(parts) / B)
    return loss, res


def kernel(logits, candidates, sampled_indices):
    loss, _ = run(logits, candidates, sampled_indices, trace=False)
    return loss


# revision 21
# speedup vs baseline: 1.0864x; 1.0530x over previous
"""AdaptiveCLPL loss on 8 TRN2 NeuronCores (Bass/Tile), v4.

loss = mean_b [ psi(avg_cand_b) + sum_head psi(-l)(1-mask) + ts*sum_samp psi(-l)(1-iscand) ]
psi(u) = softplus(-u); psi(-l) = softplus(l) = Ln(Exp(l)+1) (composite; both
funcs live in the single natural_log_exp_and_others act table -> one load).

Decomposition (host does index-driven data selection/layout only; every
logit VALUE is read, transformed and reduced on device):
  total = sum_b softplus(-avg_b)                       [term1]
        + sum_{head block} softplus(l)                 [bulk DMA + ACT/DVE]
        + ts * sum_{sampled cols, all rows} softplus(l)
        + sum_cand wcorr * softplus(l_cand),  wcorr = -uniq*(inhead + ts*smult)

Per-core layout (rows = its 256-row batch shard; batch row b lives at
partition p=b%128, group g=b//128):
- "pref" [128, 2*GMX + 2*NCP + 2*S] bf16:
    [cand g0 | cand g1]: row (p,g)'s unique non-correction candidate logits,
      padded with 0.0 (adds zero to the row-sum) -> csum = plain row reduce.
    [corr g0 | corr g1]: correction-candidate logits at their row's slot;
      also reduced into csum; softplus(pad=0)=ln2 is cancelled by wcpm=0.
    [samp h0 | samp h1]: the 100 sampled columns for all 256 rows
      (every element needed) -> softplus + accum.
- "lTh" [2000, 256] bf16 head block, viewed [125, 4096]: chunked ACT
  Exp then Ln(+1) with per-chunk row-sum accumulation (overlaps the DMA).
Per-core [128,1] partials are summed on host (no collectives, no gpsimd,
no SWDGE gathers -- see kernel_gather.py for why gathers lose: ~8.4ns/idx
serial descriptor emission + ~9us IRAM library load + first-run races).
"""

import numpy as np
import ml_dtypes

B, C, K = 2048, 50000, 10
HEAD, S = 2000, 100
TSCALE = float(C - HEAD) / float(S)  # 480.0
NCORES = 8
RB = B // NCORES   # 256
P = 128
HP = 125           # head tile partitions; 2000 = 125*16
HB = HEAD // HP    # 16
HW4 = HB * RB // 4  # head tile quarter width (1024)
HEADP = 2048       # head cols padded to 128*16 (pad value -20)
HBP = HEADP // P   # 16
BF16 = ml_dtypes.bfloat16

_CACHE = {}


def prep_inputs(logits, candidates, sampled_indices):
    logits = np.asarray(logits)
    candidates = np.asarray(candidates)
    sampled_indices = np.asarray(sampled_indices)
    assert logits.shape == (B, C) and candidates.shape == (B, K)
    srow = (HEAD + sampled_indices.astype(np.int64))      # [S] column ids
    svals, scounts = np.unique(srow, return_counts=True)
    smult_map = dict(zip(svals.tolist(), scounts.tolist()))

    cores = []
    for i in range(NCORES):
        rows = slice(i * RB, (i + 1) * RB)
        cand = candidates[rows].astype(np.int64)
        valid = cand >= 0
        uniq = valid.copy()
        for k in range(1, K):
            dup = (cand[:, :k] == cand[:, k:k + 1]).any(axis=1)
            uniq[:, k] &= ~dup
        cnt = np.maximum(uniq.sum(axis=1), 1).astype(np.float32)
        inhead = cand < HEAD
        mult = np.vectorize(lambda c: smult_map.get(int(c), 0))(cand)
        iscorr = uniq & (inhead | (mult > 0))
        plain = [[] for _ in range(RB)]   # candidate col ids per row
        corr = [[] for _ in range(RB)]    # (col, wcorr) per row
        for b in range(RB):
            for k in range(K):
                if not uniq[b, k]:
                    continue
                if iscorr[b, k]:
                    corr[b].append((int(cand[b, k]),
                                    -(float(inhead[b, k])
                                      + TSCALE * float(mult[b, k]))))
                else:
                    plain[b].append(int(cand[b, k]))
        cores.append((plain, corr, cnt))

    gmx = max(max(len(pl) for pl in plain_) or 1
              for plain_, _, _ in cores)
    ncp = max(max(len(co) for co in corr_) or 1
              for _, corr_, _ in cores)
    plan = (gmx, ncp)

    in_maps = []
    for i in range(NCORES):
        plain, corr, cnt = cores[i]
        rows = slice(i * RB, (i + 1) * RB)
        lrows = logits[rows]                              # [256, C] f32

        cv = np.zeros((RB, gmx), np.float32)
        xv = np.zeros((RB, ncp), np.float32)
        wc = np.zeros((RB, ncp), np.float32)
        for b in range(RB):
            for j, col in enumerate(plain[b]):
                cv[b, j] = lrows[b, col]
            for j, (col, w) in enumerate(corr[b]):
                xv[b, j] = lrows[b, col]
                wc[b, j] = w
        sampv = lrows[:, srow]                            # [256, S]

        def fold(a):
            """[256, W] -> [128, 2*W] with (p, g*W + j) = a[g*128+p, j]."""
            return np.concatenate([a[:128], a[128:]], axis=1)

        rcnt = np.zeros((P, 2), np.float32)
        for b in range(RB):
            rcnt[b % 128, b // 128] = 1.0 / cnt[b]
        # head padded to 2048 columns with -20 (softplus ~ 2e-9): partition
        # p holds cols [16p, 16p+16) contiguously -> [128, 4096]
        hpad = np.full((HEADP, RB), -20.0, np.float32)
        hpad[:HEAD] = lrows[:, :HEAD].T
        lTh = hpad.astype(BF16).reshape(P, HBP * RB)
        smalls = np.concatenate(
            [fold(cv), fold(xv), fold(sampv), fold(wc), rcnt], axis=1)
        spad = np.zeros((P, (-smalls.shape[1]) % 256), np.float32)
        pref = np.concatenate([smalls, spad, lTh], axis=1).astype(BF16)

        in_maps.append({
            "pref": np.ascontiguousarray(pref),
        })
    return in_maps, plan


def _build(plan, enable_asserts=False):
    import concourse.tile as tile
    from concourse import bacc, mybir

    gmx, ncp = plan
    SW0 = 2 * gmx + 2 * ncp + 2 * S + 2 * ncp + 2
    SW = SW0 + ((-SW0) % 256)
    PW = SW + HBP * RB

    f32 = mybir.dt.float32
    bf16 = mybir.dt.bfloat16
    AF = mybir.ActivationFunctionType
    OP = mybir.AluOpType
    AX = mybir.AxisListType

    nc = bacc.Bacc("TRN2", target_bir_lowering=False, debug=False,
                   enable_asserts=enable_asserts, num_devices=NCORES)

    from concourse.hw_specs import get_activation_tables
    tabs = get_activation_tables(nc.m.arch)
    if "natural_log_exp_and_others" in tabs:
        for nm, funcs in tabs.items():
            if nm != "natural_log_exp_and_others":
                funcs.discard(AF.Exp)
                funcs.discard(AF.Ln)

    pref = nc.dram_tensor("pref", [P, PW], bf16, kind="ExternalInput").ap()
    out = nc.dram_tensor("out", [P, 6], f32, kind="ExternalOutput").ap()


    with tile.TileContext(nc) as tc:
        with tc.tile_pool(name="sb", bufs=1) as sb:
            # --- ONE packed input, two half-transfers (fixed DMA cost
            # ~2.5us dominates small transfers; payload ~285GB/s) ---
            pf = sb.tile([P, PW], bf16)
            halfc = PW // 2
            nc.sync.dma_start(out=pf[:, :halfc], in_=pref[:, :halfc])
            nc.scalar.dma_start(out=pf[:, halfc:], in_=pref[:, halfc:])
            ht = pf[:, SW:PW]

            o = 0
            cand_t = pf[:, o:o + 2 * gmx]; o += 2 * gmx
            corr_t = pf[:, o:o + 2 * ncp]; o += 2 * ncp
            samp_t = pf[:, o:o + 2 * S]; o += 2 * S
            wcpm_b = pf[:, o:o + 2 * ncp]; o += 2 * ncp
            rcnt_b = pf[:, o:o + 2]; o += 2
            # f32 working copies of the bf16 weights
            wr = sb.tile([P, 2 * ncp + 2], f32)
            nc.vector.tensor_scalar_mul(wr[:, :], pf[:, o - 2 * ncp - 2:o],
                                        1.0)
            wcpm_t = wr[:, 0:2 * ncp]
            rcnt_t = wr[:, 2 * ncp:2 * ncp + 2]

            # --- sampled: softplus + accum (bf16) ---
            sp = sb.tile([P, 2 * S], bf16)
            nc.scalar.activation(sp[:, :], samp_t, AF.Exp)
            sacc = sb.tile([P, 1], f32)
            nc.scalar.activation(sp[:, :], sp[:, :], AF.Ln, bias=1.0,
                                 accum_out=sacc[:, :])

            # --- corrections: softplus(corr values) dot wcpm ---
            ce = sb.tile([P, 2 * ncp], f32)
            nc.scalar.activation(ce[:, :], corr_t, AF.Exp)
            spl = sb.tile([P, 2 * ncp], f32)
            nc.scalar.activation(spl[:, :], ce[:, :], AF.Ln, bias=1.0)
            nc.vector.tensor_tensor(spl[:, :], spl[:, :], wcpm_t,
                                    op=OP.mult)
            corr1 = sb.tile([P, 1], f32)
            nc.vector.tensor_reduce(corr1[:, :], spl[:, :], AX.X, OP.add)

            # --- candidate row sums -> avg -> term1 ---
            csum = sb.tile([P, 2], f32)
            nc.vector.tensor_reduce(
                csum[:, :], cand_t.rearrange("p (g j) -> p g j", g=2),
                AX.X, OP.add)
            csc = sb.tile([P, 2], f32)
            nc.vector.tensor_reduce(
                csc[:, :], corr_t.rearrange("p (g j) -> p g j", g=2),
                AX.X, OP.add)
            nc.vector.tensor_tensor(csum[:, :], csum[:, :], csc[:, :],
                                    op=OP.add)
            avg = sb.tile([P, 2], f32)
            nc.vector.tensor_tensor(avg[:, :], csum[:, :], rcnt_t,
                                    op=OP.mult)
            ae = sb.tile([P, 2], f32)
            nc.scalar.activation(ae[:, :], avg[:, :], AF.Exp, scale=-1.0)
            t1 = sb.tile([P, 2], f32)
            t1c = sb.tile([P, 1], f32)
            nc.scalar.activation(t1[:, :], ae[:, :], AF.Ln, bias=1.0,
                                 accum_out=t1c[:, :])

            # --- head softplus: ACT Exp chunks; DVE adds 1 and takes
            # 8-wide products (bf16, unbiased); ACT Ln on the 512 products
            # per partition: sum ln(1+e^x) = ln prod (1+e^x) ---
            hw2 = HBP * RB // 4
            hprod = sb.tile([P, HBP * RB // 8], bf16)
            for qi in range(4):
                sl = slice(SW + qi * hw2, SW + (qi + 1) * hw2)
                nc.scalar.activation(pf[:, sl], pf[:, sl], AF.Exp)
                nc.vector.tensor_scalar_add(pf[:, sl], pf[:, sl], 1.0)
                nc.vector.tensor_reduce(
                    hprod[:, qi * hw2 // 8:(qi + 1) * hw2 // 8],
                    pf[:, sl].rearrange("p (a e) -> p a e", e=8),
                    AX.X, OP.mult)
            hl = sb.tile([P, HBP * RB // 8], f32)
            hacc4 = sb.tile([P, 1], f32)
            nc.scalar.activation(hl[:, :], hprod[:, :], AF.Ln,
                                 accum_out=hacc4[:, :])

            # --- outputs: small terms early, head accums when done;
            # the host sums the 6 columns ---
            total = sb.tile([P, 6], f32)
            nc.vector.memset(total[:, :], 0.0)
            nc.vector.tensor_scalar_mul(total[:, 0:1], sacc[:, :], TSCALE)
            nc.vector.tensor_tensor(total[:, 0:1], total[:, 0:1],
                                    t1c[:, :], op=OP.add)
            nc.vector.tensor_tensor(total[:, 0:1], total[:, 0:1],
                                    corr1[:, :], op=OP.add)
            nc.sync.dma_start(out=out[:, 0:2], in_=total[:, 0:2])
            nc.vector.tensor_tensor(total[:, 2:3], total[:, 2:3],
                                    hacc4[:, :], op=OP.add)
            nc.scalar.dma_start(out=out[:, 2:3], in_=total[:, 2:3])

    nc.compile()
    return nc


def get_graph(plan, enable_asserts=False):
    key = (plan, enable_asserts)
    if key not in _CACHE:
        _CACHE[key] = _build(plan, enable_asserts=enable_asserts)
    return _CACHE[key]


def run(logits, candidates, sampled_indices, trace=False, **kw):
    from concourse.bass_utils import run_bass_kernel_spmd

    in_maps, plan = prep_inputs(logits, candidates, sampled_indices)
    nc = get_graph(plan)
    res = run_bass_kernel_spmd(nc, in_maps, core_ids=list(range(NCORES)),
                               trace=trace, **kw)
    parts = [r["out"].astype(np.float64).sum() for r in res.results]
    loss = np.float32(sum(parts) / B)
    return loss, res


def kernel(logits, candidates, sampled_indices):
    loss, _ = run(logits, candidates, sampled_indices, trace=False)
    return loss


# revision 22
# speedup vs baseline: 1.1642x; 1.0717x over previous
"""AdaptiveCLPL loss on 8 TRN2 NeuronCores (Bass/Tile), v4.

loss = mean_b [ psi(avg_cand_b) + sum_head psi(-l)(1-mask) + ts*sum_samp psi(-l)(1-iscand) ]
psi(u) = softplus(-u); psi(-l) = softplus(l) = Ln(Exp(l)+1) (composite; both
funcs live in the single natural_log_exp_and_others act table -> one load).

Decomposition (host does index-driven data selection/layout only; every
logit VALUE is read, transformed and reduced on device):
  total = sum_b softplus(-avg_b)                       [term1]
        + sum_{head block} softplus(l)                 [bulk DMA + ACT/DVE]
        + ts * sum_{sampled cols, all rows} softplus(l)
        + sum_cand wcorr * softplus(l_cand),  wcorr = -uniq*(inhead + ts*smult)

Per-core layout (rows = its 256-row batch shard; batch row b lives at
partition p=b%128, group g=b//128):
- "pref" [128, 2*GMX + 2*NCP + 2*S] bf16:
    [cand g0 | cand g1]: row (p,g)'s unique non-correction candidate logits,
      padded with 0.0 (adds zero to the row-sum) -> csum = plain row reduce.
    [corr g0 | corr g1]: correction-candidate logits at their row's slot;
      also reduced into csum; softplus(pad=0)=ln2 is cancelled by wcpm=0.
    [samp h0 | samp h1]: the 100 sampled columns for all 256 rows
      (every element needed) -> softplus + accum.
- "lTh" [2000, 256] bf16 head block, viewed [125, 4096]: chunked ACT
  Exp then Ln(+1) with per-chunk row-sum accumulation (overlaps the DMA).
Per-core [128,1] partials are summed on host (no collectives, no gpsimd,
no SWDGE gathers -- see kernel_gather.py for why gathers lose: ~8.4ns/idx
serial descriptor emission + ~9us IRAM library load + first-run races).
"""

import numpy as np
import ml_dtypes

B, C, K = 2048, 50000, 10
HEAD, S = 2000, 100
TSCALE = float(C - HEAD) / float(S)  # 480.0
NCORES = 8
RB = B // NCORES   # 256
P = 128
HP = 125           # head tile partitions; 2000 = 125*16
HB = HEAD // HP    # 16
HW4 = HB * RB // 4  # head tile quarter width (1024)
HEADP = 2048       # head cols padded to 128*16 (pad value -20)
HBP = HEADP // P   # 16
BF16 = ml_dtypes.bfloat16

_CACHE = {}


def prep_inputs(logits, candidates, sampled_indices):
    logits = np.asarray(logits)
    candidates = np.asarray(candidates)
    sampled_indices = np.asarray(sampled_indices)
    assert logits.shape == (B, C) and candidates.shape == (B, K)
    srow = (HEAD + sampled_indices.astype(np.int64))      # [S] column ids
    svals, scounts = np.unique(srow, return_counts=True)
    smult_map = dict(zip(svals.tolist(), scounts.tolist()))

    cores = []
    for i in range(NCORES):
        rows = slice(i * RB, (i + 1) * RB)
        cand = candidates[rows].astype(np.int64)
        valid = cand >= 0
        uniq = valid.copy()
        for k in range(1, K):
            dup = (cand[:, :k] == cand[:, k:k + 1]).any(axis=1)
            uniq[:, k] &= ~dup
        cnt = np.maximum(uniq.sum(axis=1), 1).astype(np.float32)
        inhead = cand < HEAD
        mult = np.vectorize(lambda c: smult_map.get(int(c), 0))(cand)
        iscorr = uniq & (inhead | (mult > 0))
        plain = [[] for _ in range(RB)]   # candidate col ids per row
        corr = [[] for _ in range(RB)]    # (col, wcorr) per row
        for b in range(RB):
            for k in range(K):
                if not uniq[b, k]:
                    continue
                if iscorr[b, k]:
                    corr[b].append((int(cand[b, k]),
                                    -(float(inhead[b, k])
                                      + TSCALE * float(mult[b, k]))))
                else:
                    plain[b].append(int(cand[b, k]))
        cores.append((plain, corr, cnt))

    gmx = max(max(len(pl) for pl in plain_) or 1
              for plain_, _, _ in cores)
    ncp = max(max(len(co) for co in corr_) or 1
              for _, corr_, _ in cores)
    plan = (gmx, ncp)

    in_maps = []
    for i in range(NCORES):
        plain, corr, cnt = cores[i]
        rows = slice(i * RB, (i + 1) * RB)
        lrows = logits[rows]                              # [256, C] f32

        cv = np.zeros((RB, gmx), np.float32)
        xv = np.zeros((RB, ncp), np.float32)
        wc = np.zeros((RB, ncp), np.float32)
        for b in range(RB):
            for j, col in enumerate(plain[b]):
                cv[b, j] = lrows[b, col]
            for j, (col, w) in enumerate(corr[b]):
                xv[b, j] = lrows[b, col]
                wc[b, j] = w
        sampv = lrows[:, srow]                            # [256, S]

        def fold(a):
            """[256, W] -> [128, 2*W] with (p, g*W + j) = a[g*128+p, j]."""
            return np.concatenate([a[:128], a[128:]], axis=1)

        rcnt = np.zeros((P, 2), np.float32)
        for b in range(RB):
            rcnt[b % 128, b // 128] = 1.0 / cnt[b]
        # head padded to 2048 columns with -20 (softplus ~ 2e-9): partition
        # p holds cols [16p, 16p+16) contiguously -> [128, 4096]
        hpad = np.full((HEADP, RB), -20.0, np.float32)
        hpad[:HEAD] = lrows[:, :HEAD].T
        lTh = hpad.astype(BF16).reshape(P, HBP * RB)
        smalls = np.concatenate(
            [fold(cv), fold(xv), fold(sampv), fold(wc), rcnt], axis=1)
        spad = np.zeros((P, (-smalls.shape[1]) % 256), np.float32)
        pref = np.concatenate([smalls, spad, lTh], axis=1).astype(BF16)

        in_maps.append({
            "pref": np.ascontiguousarray(pref),
        })
    return in_maps, plan


def _build(plan, enable_asserts=False):
    import concourse.tile as tile
    from concourse import bacc, mybir

    gmx, ncp = plan
    SW0 = 2 * gmx + 2 * ncp + 2 * S + 2 * ncp + 2
    SW = SW0 + ((-SW0) % 256)
    PW = SW + HBP * RB

    f32 = mybir.dt.float32
    bf16 = mybir.dt.bfloat16
    AF = mybir.ActivationFunctionType
    OP = mybir.AluOpType
    AX = mybir.AxisListType

    nc = bacc.Bacc("TRN2", target_bir_lowering=False, debug=False,
                   enable_asserts=enable_asserts, num_devices=NCORES)

    from concourse.hw_specs import get_activation_tables
    tabs = get_activation_tables(nc.m.arch)
    if "natural_log_exp_and_others" in tabs:
        for nm, funcs in tabs.items():
            if nm != "natural_log_exp_and_others":
                funcs.discard(AF.Exp)
                funcs.discard(AF.Ln)

    pref = nc.dram_tensor("pref", [P, PW], bf16, kind="ExternalInput").ap()
    out = nc.dram_tensor("out", [P, 6], f32, kind="ExternalOutput").ap()


    with tile.TileContext(nc) as tc:
        with tc.tile_pool(name="sb", bufs=1) as sb:
            # --- ONE packed input, two half-transfers (fixed DMA cost
            # ~2.5us dominates small transfers; payload ~285GB/s) ---
            pf = sb.tile([P, PW], bf16)
            halfc = PW // 2
            nc.sync.dma_start(out=pf[:, :halfc], in_=pref[:, :halfc])
            nc.scalar.dma_start(out=pf[:, halfc:], in_=pref[:, halfc:])
            ht = pf[:, SW:PW]

            o = 0
            cand_t = pf[:, o:o + 2 * gmx]; o += 2 * gmx
            corr_t = pf[:, o:o + 2 * ncp]; o += 2 * ncp
            samp_t = pf[:, o:o + 2 * S]; o += 2 * S
            wcpm_b = pf[:, o:o + 2 * ncp]; o += 2 * ncp
            rcnt_b = pf[:, o:o + 2]; o += 2
            # f32 working copies of the bf16 weights
            wr = sb.tile([P, 2 * ncp + 2], f32)
            nc.vector.tensor_scalar_mul(wr[:, :], pf[:, o - 2 * ncp - 2:o],
                                        1.0)
            wcpm_t = wr[:, 0:2 * ncp]
            rcnt_t = wr[:, 2 * ncp:2 * ncp + 2]

            # --- head softplus: ACT Exp chunks; DVE adds 1 and takes
            # 8-wide products (bf16, unbiased); ACT Ln on the 512 products
            # per partition: sum ln(1+e^x) = ln prod (1+e^x) ---
            hw2 = HBP * RB // 4
            hprod = sb.tile([P, HBP * RB // 8], bf16)
            for qi in range(4):
                sl = slice(SW + qi * hw2, SW + (qi + 1) * hw2)
                nc.scalar.activation(pf[:, sl], pf[:, sl], AF.Exp)
                nc.vector.tensor_scalar_add(pf[:, sl], pf[:, sl], 1.0)
                nc.vector.tensor_reduce(
                    hprod[:, qi * hw2 // 8:(qi + 1) * hw2 // 8],
                    pf[:, sl].rearrange("p (a e) -> p a e", e=8),
                    AX.X, OP.mult)
            hl = sb.tile([P, HBP * RB // 8], f32)
            hacc4 = sb.tile([P, 1], f32)
            nc.scalar.activation(hl[:, :], hprod[:, :], AF.Ln,
                                 accum_out=hacc4[:, :])

            # --- sampled: softplus + accum (bf16) ---
            sp = sb.tile([P, 2 * S], bf16)
            nc.scalar.activation(sp[:, :], samp_t, AF.Exp)
            sacc = sb.tile([P, 1], f32)
            nc.scalar.activation(sp[:, :], sp[:, :], AF.Ln, bias=1.0,
                                 accum_out=sacc[:, :])

            # --- corrections: softplus(corr values) dot wcpm ---
            ce = sb.tile([P, 2 * ncp], f32)
            nc.scalar.activation(ce[:, :], corr_t, AF.Exp)
            spl = sb.tile([P, 2 * ncp], f32)
            nc.scalar.activation(spl[:, :], ce[:, :], AF.Ln, bias=1.0)
            nc.vector.tensor_tensor(spl[:, :], spl[:, :], wcpm_t,
                                    op=OP.mult)
            corr1 = sb.tile([P, 1], f32)
            nc.vector.tensor_reduce(corr1[:, :], spl[:, :], AX.X, OP.add)

            # --- candidate row sums -> avg -> term1 ---
            csum = sb.tile([P, 2], f32)
            nc.vector.tensor_reduce(
                csum[:, :], cand_t.rearrange("p (g j) -> p g j", g=2),
                AX.X, OP.add)
            csc = sb.tile([P, 2], f32)
            nc.vector.tensor_reduce(
                csc[:, :], corr_t.rearrange("p (g j) -> p g j", g=2),
                AX.X, OP.add)
            nc.vector.tensor_tensor(csum[:, :], csum[:, :], csc[:, :],
                                    op=OP.add)
            avg = sb.tile([P, 2], f32)
            nc.vector.tensor_tensor(avg[:, :], csum[:, :], rcnt_t,
                                    op=OP.mult)
            ae = sb.tile([P, 2], f32)
            nc.scalar.activation(ae[:, :], avg[:, :], AF.Exp, scale=-1.0)
            t1 = sb.tile([P, 2], f32)
            t1c = sb.tile([P, 1], f32)
            nc.scalar.activation(t1[:, :], ae[:, :], AF.Ln, bias=1.0,
                                 accum_out=t1c[:, :])

            # --- outputs: small terms early, head accums when done;
            # the host sums the 6 columns ---
            total = sb.tile([P, 6], f32)
            nc.vector.memset(total[:, :], 0.0)
            nc.vector.tensor_scalar_mul(total[:, 0:1], sacc[:, :], TSCALE)
            nc.vector.tensor_tensor(total[:, 0:1], total[:, 0:1],
                                    t1c[:, :], op=OP.add)
            nc.vector.tensor_tensor(total[:, 0:1], total[:, 0:1],
                                    corr1[:, :], op=OP.add)
            nc.sync.dma_start(out=out[:, 0:2], in_=total[:, 0:2])
            nc.vector.tensor_tensor(total[:, 2:3], total[:, 2:3],
                                    hacc4[:, :], op=OP.add)
            nc.scalar.dma_start(out=out[:, 2:3], in_=total[:, 2:3])

    nc.compile()
    return nc


def get_graph(plan, enable_asserts=False):
    key = (plan, enable_asserts)
    if key not in _CACHE:
        _CACHE[key] = _build(plan, enable_asserts=enable_asserts)
    return _CACHE[key]


def run(logits, candidates, sampled_indices, trace=False, **kw):
    from concourse.bass_utils import run_bass_kernel_spmd

    in_maps, plan = prep_inputs(logits, candidates, sampled_indices)
    nc = get_graph(plan)
    res = run_bass_kernel_spmd(nc, in_maps, core_ids=list(range(NCORES)),
                               trace=trace, **kw)
    parts = [r["out"].astype(np.float64).sum() for r in res.results]
    loss = np.float32(sum(parts) / B)
    return loss, res


def kernel(logits, candidates, sampled_indices):
    loss, _ = run(logits, candidates, sampled_indices, trace=False)
    return loss
